# revision 11
# baseline (speedup 1.0000x reference)
"""Trainium2 Bass kernel for nn_AttentionEncoder (luong_concat + asee coverage).

Computes, for each batch b:
    agg   = tanh(enhy @ W_en.T + b_en + dehy @ W_de.T + past_attn[:,None]*w_cv)
    ee    = agg @ w_warp, masked_fill(src_mask==0, -1e20)
    attn  = softmax(ee)
    c     = attn @ enhy
Returns (c_encoder, attn, attn_ee).

Sharding: data-parallel over batch B=64 across 8 NeuronCores (8 batches/core),
weights replicated. All matmuls run in float32r (TF32-like, 1 cycle/row) with
fp32 PSUM accumulation.
"""
import numpy as np
from contextlib import ExitStack

import concourse.bass as bass
import concourse.mybir as mybir
import concourse.tile as tile
from concourse import bacc
from concourse.bass_utils import run_bass_kernel_spmd
from concourse.masks import make_identity

B, S, TRG, SRC2 = 64, 2048, 512, 1024
N_CORES = 8
BPC = B // N_CORES   # batches per core
SB = S // 128        # 16 s-blocks
DB = SRC2 // 128     # 8 d-blocks
HB = TRG // 128      # 4 h-blocks
F32 = mybir.dt.float32
F32R = mybir.dt.float32r
I32 = mybir.dt.int32
AF = mybir.ActivationFunctionType
ALU = mybir.AluOpType


def build_nc(stage="full"):
    nc = bacc.Bacc("TRN2", target_bir_lowering=False, debug=False, enable_asserts=True)

    dehy_d = nc.dram_tensor("dehy", [BPC, TRG], F32, kind="ExternalInput").ap()
    enhy_d = nc.dram_tensor("enhy", [BPC, S, SRC2], F32, kind="ExternalInput").ap()
    pa_d = nc.dram_tensor("past_attn", [BPC, S], F32, kind="ExternalInput").ap()
    mask_d = nc.dram_tensor("src_mask", [BPC, S], I32, kind="ExternalInput").ap()
    wen_d = nc.dram_tensor("W_en", [TRG, SRC2], F32, kind="ExternalInput").ap()
    ben_d = nc.dram_tensor("b_en", [TRG], F32, kind="ExternalInput").ap()
    wde_d = nc.dram_tensor("W_de", [TRG, TRG], F32, kind="ExternalInput").ap()
    wcv_d = nc.dram_tensor("w_cv", [TRG], F32, kind="ExternalInput").ap()
    wwarp_d = nc.dram_tensor("w_warp", [TRG], F32, kind="ExternalInput").ap()
    cenc_d = nc.dram_tensor("c_encoder", [BPC, SRC2], F32, kind="ExternalOutput").ap()
    attn_d = nc.dram_tensor("attn", [BPC, S], F32, kind="ExternalOutput").ap()
    aee_d = nc.dram_tensor("attn_ee", [BPC, S], F32, kind="ExternalOutput").ap()

    with tile.TileContext(nc) as tc, ExitStack() as ctx:
        const_pool = ctx.enter_context(tc.tile_pool(name="const", bufs=1))
        wt_pool = ctx.enter_context(tc.tile_pool(name="wt", bufs=1))

        # ---- constants ----
        id_f = const_pool.tile([128, 128], F32, tag="id_f")
        make_identity(nc, id_f[:])
        id_r = const_pool.tile([128, 128], F32R, tag="id_r")
        nc.vector.tensor_copy(id_r[:], id_f[:])
        ones_f = const_pool.tile([1, 128], F32, tag="ones_f")
        nc.gpsimd.memset(ones_f[:], 1.0)
        ones_r = const_pool.tile([1, 128], F32R, tag="ones_r")
        nc.vector.tensor_copy(ones_r[:], ones_f[:])
        ones_col_f = const_pool.tile([128, 2], F32, tag="ones_col_f")
        nc.gpsimd.memset(ones_col_f[:], 1.0)
        ones_col2_r = const_pool.tile([128, 2], F32R, tag="ones_col2_r")
        nc.vector.tensor_copy(ones_col2_r[:], ones_col_f[:])
        ones_row = const_pool.tile([1, S], F32, tag="ones_row")
        nc.gpsimd.memset(ones_row[:], 1.0)

        # ---- persistent weights/data ----
        wenT = wt_pool.tile([128, DB, TRG], F32R, tag="wenT")       # [d_p, db, h]
        bias2 = wt_pool.tile([2, BPC, TRG], F32R, tag="bias2")      # row0 w_cv, row1 b_en+dehy@W_de.T
        ww_bc = wt_pool.tile([128, TRG], F32, tag="ww_bc")          # w_warp broadcast over partitions
        maskf = wt_pool.tile([128, SB, BPC], F32, tag="maskf")      # [p, j, b] mask as float
        negoff = wt_pool.tile([128, SB, BPC], F32, tag="negoff")    # (mask-1)*1e20
        scores = wt_pool.tile([128, BPC * SB], F32, tag="scores")

        # ---- setup (transient pools) ----
        with tc.tile_pool(name="setup", bufs=1) as sp, \
             tc.tile_pool(name="setup_ps", bufs=2, space="PSUM") as sps:
            # W_en -> W_enT (fp32r)
            wen_stage = sp.tile([128, HB, SRC2], F32R, tag="wen_stage")
            nc.gpsimd.dma_start(out=wen_stage[:],
                                in_=wen_d.rearrange("(hb p) d -> p hb d", p=128))
            for db in range(DB):
                pt = sps.tile([128, HB, 128], F32R, tag="s_pt")
                for hb in range(HB):
                    nc.tensor.transpose(pt[:, hb], wen_stage[:, hb, db * 128:(db + 1) * 128], id_r[:])
                nc.vector.tensor_copy(wenT[:, db], pt[:])

            # W_de -> W_deT (fp32r)  [t_p, tb, h]
            wde_stage = sp.tile([128, HB, TRG], F32R, tag="wde_stage")
            nc.gpsimd.dma_start(out=wde_stage[:],
                                in_=wde_d.rearrange("(hb p) t -> p hb t", p=128))
            wdeT = sp.tile([128, HB, TRG], F32R, tag="wdeT")
            for tb in range(HB):
                pt = sps.tile([128, HB, 128], F32R, tag="s_pt")
                for hb in range(HB):
                    nc.tensor.transpose(pt[:, hb], wde_stage[:, hb, tb * 128:(tb + 1) * 128], id_r[:])
                nc.vector.tensor_copy(wdeT[:, tb], pt[:])

            # dehy -> dehyT
            dehy_stage = sp.tile([BPC, TRG], F32R, tag="dehy_stage")
            nc.gpsimd.dma_start(out=dehy_stage[:], in_=dehy_d[:])
            dehyT = sp.tile([128, HB, BPC], F32R, tag="dehyT")
            for tb in range(HB):
                ptd = sps.tile([128, BPC], F32R, tag="s_pt")
                nc.tensor.transpose(ptd[:], dehy_stage[:, tb * 128:(tb + 1) * 128], id_r[:BPC, :BPC])
                nc.vector.tensor_copy(dehyT[:, tb], ptd[:])

            # bias = b_en + dehy @ W_de.T   -> [BPC, TRG]
            bias_ps = sps.tile([BPC, TRG], F32, tag="s_misc")
            for tb in range(HB):
                nc.tensor.matmul(bias_ps[:], dehyT[:, tb], wdeT[:, tb],
                                 start=(tb == 0), stop=False)
            ben_sb = sp.tile([1, TRG], F32R, tag="ben")
            nc.gpsimd.dma_start(out=ben_sb[:], in_=ben_d.rearrange("(a t) -> a t", a=1))
            nc.tensor.matmul(bias_ps[:], ones_r[:1, :BPC], ben_sb[:], start=False, stop=True)
            bias_stage = sp.tile([BPC, TRG], F32R, tag="bias_stage")
            nc.vector.tensor_copy(bias_stage[:], bias_ps[:])
            for b in range(BPC):
                nc.gpsimd.dma_start(out=bias2[0:1, b], in_=wcv_d.rearrange("(a t) -> a t", a=1))
                nc.gpsimd.dma_start(out=bias2[1:2, b], in_=bias_stage[b:b + 1, :])

            # w_warp broadcast to all partitions
            ww_sb = sp.tile([1, TRG], F32R, tag="ww")
            nc.gpsimd.dma_start(out=ww_sb[:], in_=wwarp_d.rearrange("(a t) -> a t", a=1))
            ww_ps = sps.tile([128, TRG], F32, tag="s_misc")
            nc.tensor.matmul(ww_ps[:], ones_r[:], ww_sb[:], start=True, stop=True)
            nc.vector.tensor_copy(ww_bc[:], ww_ps[:])

            # masks: natural int32 load, value-cast to float, PE-transpose to [p, j, b]
            mask_i = sp.tile([BPC, S], I32, tag="mask_i")
            nc.gpsimd.dma_start(out=mask_i[:], in_=mask_d[:])
            mask_natf = sp.tile([BPC, S], F32, tag="mask_natf")
            nc.vector.tensor_copy(mask_natf[:], mask_i[:])   # int -> float value cast
            for j in range(SB):
                pm = sps.tile([128, BPC], F32, tag="s_misc")
                nc.tensor.transpose(pm[:], mask_natf[:, j * 128:(j + 1) * 128], id_f[:BPC, :BPC])
                nc.vector.tensor_copy(maskf[:, j], pm[:])
            nc.vector.tensor_scalar(out=negoff[:], in0=maskf[:], scalar1=1.0,
                                    scalar2=1e20, op0=ALU.subtract, op1=ALU.mult)

        # ---- batch pools ----
        enat_pool = ctx.enter_context(tc.tile_pool(name="enat", bufs=3))
        etr_pool = ctx.enter_context(tc.tile_pool(name="etr", bufs=2))
        work_pool = ctx.enter_context(tc.tile_pool(name="work", bufs=2))
        out_pool = ctx.enter_context(tc.tile_pool(name="outp", bufs=2))
        pa_pool = ctx.enter_context(tc.tile_pool(name="pa", bufs=2))
        ps_t = ctx.enter_context(tc.tile_pool(name="ps_t", bufs=2, space="PSUM"))
        ps_x = ctx.enter_context(tc.tile_pool(name="ps_x", bufs=2, space="PSUM"))
        ps_c = ctx.enter_context(tc.tile_pool(name="ps_c", bufs=1, space="PSUM"))
        ps_s = ctx.enter_context(tc.tile_pool(name="ps_s", bufs=2, space="PSUM"))

        for b in range(BPC):
            src = enhy_d[b].rearrange("(j p) d -> p j d", p=128)
            ehalves = []
            for h in range(2):
                eh = enat_pool.tile([128, SB // 2, SRC2], F32R, tag="enat")
                for q in range(2):
                    lo = h * 8 + q * 4
                    nc.gpsimd.dma_start(out=eh[:, q * 4:(q + 1) * 4, :],
                                        in_=src[:, lo:lo + 4, :])
                ehalves.append(eh)

            pa2 = pa_pool.tile([2, S], F32R, tag="pa2")
            nc.gpsimd.dma_start(out=pa2[0:1, :], in_=pa_d[b:b + 1, :])
            nc.gpsimd.dma_start(out=pa2[1:2, :], in_=ones_row[:])

            if stage == "load":
                continue
            for j in range(SB):
                eh, jj = ehalves[j // 8], j % 8
                etr = etr_pool.tile([128, DB, 128], F32R, tag="etr")
                for hd in range(2):
                    pt = ps_t.tile([128, 4, 128], F32R, tag="pt")
                    for k in range(4):
                        db = hd * 4 + k
                        nc.tensor.transpose(pt[:, k], eh[:, jj, db * 128:(db + 1) * 128], id_r[:])
                    nc.vector.tensor_copy(etr[:, hd * 4:(hd + 1) * 4, :], pt[:])

                if stage == "transpose":
                    continue
                px = ps_x.tile([128, TRG], F32, tag="px")
                for db in range(DB):
                    nc.tensor.matmul(px[:], etr[:, db], wenT[:, db],
                                     start=(db == 0), stop=False)
                if stage != "mm":
                    nc.tensor.matmul(px[:], pa2[:, j * 128:(j + 1) * 128], bias2[:, b],
                                     start=False, stop=True)
                else:
                    nc.tensor.matmul(px[:], etr[:, 0], wenT[:, 0], start=False, stop=True)

                tj = work_pool.tile([128, TRG], F32, tag="tanh")
                nc.scalar.activation(tj[:], px[:], AF.Tanh)
                if stage in ("mm", "mm_rank2"):
                    continue
                ttro = work_pool.tile([128, TRG], F32, tag="ttro")
                nc.vector.tensor_mul(ttro[:], tj[:], ww_bc[:])
                nc.vector.reduce_sum(scores[:, b * SB + j: b * SB + j + 1], ttro[:],
                                     axis=mybir.AxisListType.X)

            if stage in ("transpose", "scores", "mm", "mm_rank2"):
                continue
            # ---- mask + softmax (no max subtraction: |scores| <= sum|w_warp|) ----
            sc_b = scores[:, b * SB:(b + 1) * SB]
            aee = out_pool.tile([128, SB], F32, tag="aee")
            nc.vector.tensor_mul(aee[:], sc_b, maskf[:, :, b])
            nc.vector.tensor_add(aee[:], aee[:], negoff[:, :, b])

            expv = out_pool.tile([128, SB], F32, tag="expv")
            zpart = out_pool.tile([128, 1], F32, tag="zpart")
            nc.scalar.activation(expv[:], aee[:], AF.Exp, accum_out=zpart[:])
            expr = out_pool.tile([128, SB], F32R, tag="expr")
            nc.vector.tensor_copy(expr[:], expv[:])
            zr = out_pool.tile([128, 1], F32R, tag="zr")
            nc.vector.tensor_copy(zr[:], zpart[:])

            pz = ps_s.tile([128, 128], F32, tag="small")
            # fp32r matmul needs even innermost dims: use N=2 ones columns
            nc.tensor.matmul(pz[:1, :2], zr[:], ones_col2_r[:], start=True, stop=True)
            zsb = out_pool.tile([1, 1], F32, tag="zsb")
            nc.vector.tensor_copy(zsb[:], pz[:1, :1])
            rz = out_pool.tile([1, 1], F32, tag="rz")
            nc.vector.reciprocal(rz[:], zsb[:])
            rz2_r = out_pool.tile([1, 2], F32R, tag="rz2_r")
            nc.vector.tensor_copy(rz2_r[:, 0:1], rz[:])
            nc.vector.tensor_copy(rz2_r[:, 1:2], rz[:])
            prz = ps_s.tile([128, 128], F32, tag="small")
            nc.tensor.matmul(prz[:, :2], ones_r[:], rz2_r[:], start=True, stop=True)
            rzcol = out_pool.tile([128, 1], F32, tag="rzcol")
            nc.vector.tensor_copy(rzcol[:], prz[:, :1])

            attn_col = out_pool.tile([128, SB], F32, tag="attn_col")
            nc.vector.tensor_scalar_mul(attn_col[:], expv[:], rzcol[:])

            # ---- outputs attn_ee / attn (transpose to row layout) ----
            pee = ps_s.tile([128, 128], F32, tag="small")
            nc.tensor.transpose(pee[:SB, :], aee[:], id_f[:])
            eeT = out_pool.tile([SB, 128], F32, tag="eeT")
            nc.vector.tensor_copy(eeT[:], pee[:SB, :])
            nc.gpsimd.dma_start(out=aee_d[b].rearrange("(j c) -> j c", j=SB), in_=eeT[:])

            patn = ps_s.tile([128, 128], F32, tag="small")
            nc.tensor.transpose(patn[:SB, :], attn_col[:], id_f[:])
            attnT = out_pool.tile([SB, 128], F32, tag="attnT")
            nc.vector.tensor_copy(attnT[:], patn[:SB, :])
            nc.gpsimd.dma_start(out=attn_d[b].rearrange("(j c) -> j c", j=SB), in_=attnT[:])

            if stage == "softmax":
                continue
            # ---- pass 2: c = (exp @ enhy) / Z ----
            pc = ps_c.tile([1, SRC2], F32, tag="pc")
            for j in range(SB):
                eh, jj = ehalves[j // 8], j % 8
                for dh in range(2):
                    nc.tensor.matmul(pc[:, dh * 512:(dh + 1) * 512],
                                     expr[:, j:j + 1], eh[:, jj, dh * 512:(dh + 1) * 512],
                                     start=(j == 0), stop=(j == SB - 1))
            csb = out_pool.tile([1, SRC2], F32, tag="csb")
            nc.scalar.activation(csb[:], pc[:], AF.Copy, scale=rz[:])
            nc.gpsimd.dma_start(out=cenc_d[b:b + 1, :], in_=csb[:])

    nc.compile()
    return nc


_NC_CACHE = None


def _get_nc():
    global _NC_CACHE
    if _NC_CACHE is None:
        _NC_CACHE = build_nc()
    return _NC_CACHE


def kernel(dehy, enhy, past_attn, src_mask, W_en, b_en, W_de, w_cv, w_warp):
    dehy = np.ascontiguousarray(np.asarray(dehy, dtype=np.float32))
    enhy = np.ascontiguousarray(np.asarray(enhy, dtype=np.float32))
    past_attn = np.ascontiguousarray(np.asarray(past_attn, dtype=np.float32))
    src_mask = np.ascontiguousarray(np.asarray(src_mask, dtype=np.int32))
    W_en = np.ascontiguousarray(np.asarray(W_en, dtype=np.float32))
    b_en = np.ascontiguousarray(np.asarray(b_en, dtype=np.float32))
    W_de = np.ascontiguousarray(np.asarray(W_de, dtype=np.float32))
    w_cv = np.ascontiguousarray(np.asarray(w_cv, dtype=np.float32))
    w_warp = np.ascontiguousarray(np.asarray(w_warp, dtype=np.float32))

    nc = _get_nc()
    in_maps = []
    for c in range(N_CORES):
        sl = slice(c * BPC, (c + 1) * BPC)
        in_maps.append({
            "dehy": dehy[sl], "enhy": enhy[sl], "past_attn": past_attn[sl],
            "src_mask": src_mask[sl], "W_en": W_en, "b_en": b_en,
            "W_de": W_de, "w_cv": w_cv, "w_warp": w_warp,
        })
    res = run_bass_kernel_spmd(nc, in_maps, core_ids=list(range(N_CORES)))
    c_encoder = np.concatenate([res.results[c]["c_encoder"] for c in range(N_CORES)], axis=0)
    attn = np.concatenate([res.results[c]["attn"] for c in range(N_CORES)], axis=0)
    attn_ee = np.concatenate([res.results[c]["attn_ee"] for c in range(N_CORES)], axis=0)
    return (c_encoder, attn, attn_ee)


if __name__ == "__main__":
    rng = np.random.default_rng(0)
    outs = kernel(
        dehy=rng.standard_normal((B, TRG)).astype(np.float32),
        enhy=rng.standard_normal((B, S, SRC2)).astype(np.float32),
        past_attn=rng.random((B, S)).astype(np.float32),
        src_mask=rng.integers(0, 2, (B, S)).astype(np.int32),
        W_en=(rng.standard_normal((TRG, SRC2)) * 0.02).astype(np.float32),
        b_en=np.zeros((TRG,), np.float32),
        W_de=(rng.standard_normal((TRG, TRG)) * 0.02).astype(np.float32),
        w_cv=(rng.standard_normal((TRG,)) * 0.02).astype(np.float32),
        w_warp=(rng.standard_normal((TRG,)) * 0.02).astype(np.float32),
    )
    for name, o in zip(["c_encoder", "attn", "attn_ee"], outs):
        print(name, o.shape, o.dtype, float(np.abs(o).mean()))


# revision 20
# speedup vs baseline: 21894.6080x; 21894.6080x over previous
"""Trainium2 Bass kernel for nn_AttentionEncoder (luong_concat + asee coverage).

Computes, for each batch b:
    agg   = tanh(enhy @ W_en.T + b_en + dehy @ W_de.T + past_attn[:,None]*w_cv)
    ee    = agg @ w_warp, masked_fill(src_mask==0, -1e20)
    attn  = softmax(ee)
    c     = attn @ enhy
Returns (c_encoder, attn, attn_ee).

Sharding: data-parallel over batch B=64 across 8 NeuronCores (8 batches/core),
weights replicated. All matmuls run in float32r (TF32-like, 1 cycle/row) with
fp32 PSUM accumulation.
"""
import numpy as np
from contextlib import ExitStack

import concourse.bass as bass
import concourse.mybir as mybir
import concourse.tile as tile
from concourse import bacc
from concourse.bass_utils import run_bass_kernel_spmd
from concourse.masks import make_identity

B, S, TRG, SRC2 = 64, 2048, 512, 1024
N_CORES = 8
BPC = B // N_CORES   # batches per core
SB = S // 128        # 16 s-blocks
DB = SRC2 // 128     # 8 d-blocks
HB = TRG // 128      # 4 h-blocks
F32 = mybir.dt.float32
F32R = mybir.dt.float32r
F16 = mybir.dt.float16
I32 = mybir.dt.int32
AF = mybir.ActivationFunctionType
ALU = mybir.AluOpType


def build_nc(stage="full", loop_reps=1):
    nc = bacc.Bacc("TRN2", target_bir_lowering=False, debug=False, enable_asserts=True)

    dehy_d = nc.dram_tensor("dehy", [BPC, TRG], F32, kind="ExternalInput").ap()
    enhy_d = nc.dram_tensor("enhy", [BPC, S, SRC2], F32, kind="ExternalInput").ap()
    pa_d = nc.dram_tensor("past_attn", [BPC, S], F32, kind="ExternalInput").ap()
    mask_d = nc.dram_tensor("src_mask", [BPC, S], I32, kind="ExternalInput").ap()
    wen_d = nc.dram_tensor("W_en", [TRG, SRC2], F32, kind="ExternalInput").ap()
    ben_d = nc.dram_tensor("b_en", [TRG], F32, kind="ExternalInput").ap()
    wde_d = nc.dram_tensor("W_de", [TRG, TRG], F32, kind="ExternalInput").ap()
    wcv_d = nc.dram_tensor("w_cv", [TRG], F32, kind="ExternalInput").ap()
    wwarp_d = nc.dram_tensor("w_warp", [TRG], F32, kind="ExternalInput").ap()
    cenc_d = nc.dram_tensor("c_encoder", [BPC, SRC2], F32, kind="ExternalOutput").ap()
    attn_d = nc.dram_tensor("attn", [BPC, S], F32, kind="ExternalOutput").ap()
    aee_d = nc.dram_tensor("attn_ee", [BPC, S], F32, kind="ExternalOutput").ap()

    with tile.TileContext(nc) as tc, ExitStack() as ctx:
        const_pool = ctx.enter_context(tc.tile_pool(name="const", bufs=1))
        wt_pool = ctx.enter_context(tc.tile_pool(name="wt", bufs=1))

        # ---- constants ----
        id_f = const_pool.tile([128, 128], F32, tag="id_f")
        make_identity(nc, id_f[:])
        id_r = const_pool.tile([128, 128], F32R, tag="id_r")
        nc.vector.tensor_copy(id_r[:], id_f[:])
        id_h = const_pool.tile([128, 128], F16, tag="id_h")
        nc.vector.tensor_copy(id_h[:], id_f[:])
        ones_f = const_pool.tile([1, 128], F32, tag="ones_f")
        nc.gpsimd.memset(ones_f[:], 1.0)
        ones_r = const_pool.tile([1, 128], F32R, tag="ones_r")
        nc.vector.tensor_copy(ones_r[:], ones_f[:])
        ones_col_f = const_pool.tile([128, 2], F32, tag="ones_col_f")
        nc.gpsimd.memset(ones_col_f[:], 1.0)
        ones_col2_r = const_pool.tile([128, 2], F32R, tag="ones_col2_r")
        nc.vector.tensor_copy(ones_col2_r[:], ones_col_f[:])
        ones_row = const_pool.tile([1, S], F32, tag="ones_row")
        nc.gpsimd.memset(ones_row[:], 1.0)

        # ---- persistent weights/data ----
        wenT = wt_pool.tile([128, DB, TRG], F16, tag="wenT")       # [d_p, db, h]
        bias2 = wt_pool.tile([2, BPC, TRG], F16, tag="bias2")      # row0 w_cv, row1 b_en+dehy@W_de.T
        ww_bc = wt_pool.tile([128, TRG], F32, tag="ww_bc")          # w_warp broadcast over partitions
        maskf = wt_pool.tile([128, SB, BPC], F32, tag="maskf")      # [p, j, b] mask as float
        negoff = wt_pool.tile([128, SB, BPC], F32, tag="negoff")    # (mask-1)*1e20
        scores = wt_pool.tile([128, BPC * SB], F32, tag="scores")

        # ---- setup (transient pools) ----
        with tc.tile_pool(name="setup", bufs=1) as sp, \
             tc.tile_pool(name="setup_ps", bufs=2, space="PSUM") as sps:
            # W_en -> W_enT (fp32r)
            wen_stage = sp.tile([128, HB, SRC2], F16, tag="wen_stage")
            nc.gpsimd.dma_start(out=wen_stage[:],
                                in_=wen_d.rearrange("(hb p) d -> p hb d", p=128))
            for db in range(DB):
                pt = sps.tile([128, HB, 128], F16, tag="s_pth")
                for hb in range(HB):
                    nc.tensor.transpose(pt[:, hb], wen_stage[:, hb, db * 128:(db + 1) * 128], id_h[:])
                nc.vector.tensor_copy(wenT[:, db], pt[:])

            # W_de -> W_deT (fp32r)  [t_p, tb, h]
            wde_stage = sp.tile([128, HB, TRG], F32R, tag="wde_stage")
            nc.gpsimd.dma_start(out=wde_stage[:],
                                in_=wde_d.rearrange("(hb p) t -> p hb t", p=128))
            wdeT = sp.tile([128, HB, TRG], F32R, tag="wdeT")
            for tb in range(HB):
                pt = sps.tile([128, HB, 128], F32R, tag="s_pt")
                for hb in range(HB):
                    nc.tensor.transpose(pt[:, hb], wde_stage[:, hb, tb * 128:(tb + 1) * 128], id_r[:])
                nc.vector.tensor_copy(wdeT[:, tb], pt[:])

            # dehy -> dehyT
            dehy_stage = sp.tile([BPC, TRG], F32R, tag="dehy_stage")
            nc.gpsimd.dma_start(out=dehy_stage[:], in_=dehy_d[:])
            dehyT = sp.tile([128, HB, BPC], F32R, tag="dehyT")
            for tb in range(HB):
                ptd = sps.tile([128, BPC], F32R, tag="s_pt")
                nc.tensor.transpose(ptd[:], dehy_stage[:, tb * 128:(tb + 1) * 128], id_r[:BPC, :BPC])
                nc.vector.tensor_copy(dehyT[:, tb], ptd[:])

            # bias = b_en + dehy @ W_de.T   -> [BPC, TRG]
            bias_ps = sps.tile([BPC, TRG], F32, tag="s_misc")
            for tb in range(HB):
                nc.tensor.matmul(bias_ps[:], dehyT[:, tb], wdeT[:, tb],
                                 start=(tb == 0), stop=False)
            ben_sb = sp.tile([1, TRG], F32R, tag="ben")
            nc.gpsimd.dma_start(out=ben_sb[:], in_=ben_d.rearrange("(a t) -> a t", a=1))
            nc.tensor.matmul(bias_ps[:], ones_r[:1, :BPC], ben_sb[:], start=False, stop=True)
            bias_stage = sp.tile([BPC, TRG], F16, tag="bias_stage")
            nc.vector.tensor_copy(bias_stage[:], bias_ps[:])
            for b in range(BPC):
                nc.gpsimd.dma_start(out=bias2[0:1, b], in_=wcv_d.rearrange("(a t) -> a t", a=1))
                nc.gpsimd.dma_start(out=bias2[1:2, b], in_=bias_stage[b:b + 1, :])

            # w_warp broadcast to all partitions
            ww_sb = sp.tile([1, TRG], F32R, tag="ww")
            nc.gpsimd.dma_start(out=ww_sb[:], in_=wwarp_d.rearrange("(a t) -> a t", a=1))
            ww_ps = sps.tile([128, TRG], F32, tag="s_misc")
            nc.tensor.matmul(ww_ps[:], ones_r[:], ww_sb[:], start=True, stop=True)
            nc.vector.tensor_copy(ww_bc[:], ww_ps[:])

            # masks: natural int32 load, value-cast to float, PE-transpose to [p, j, b]
            mask_i = sp.tile([BPC, S], I32, tag="mask_i")
            nc.gpsimd.dma_start(out=mask_i[:], in_=mask_d[:])
            mask_natf = sp.tile([BPC, S], F32, tag="mask_natf")
            nc.vector.tensor_copy(mask_natf[:], mask_i[:])   # int -> float value cast
            for j in range(SB):
                pm = sps.tile([128, BPC], F32, tag="s_misc")
                nc.tensor.transpose(pm[:], mask_natf[:, j * 128:(j + 1) * 128], id_f[:BPC, :BPC])
                nc.vector.tensor_copy(maskf[:, j], pm[:])
            nc.vector.tensor_scalar(out=negoff[:], in0=maskf[:], scalar1=1.0,
                                    scalar2=1e20, op0=ALU.subtract, op1=ALU.mult)

        # ---- batch pools ----
        enat_pool = ctx.enter_context(tc.tile_pool(name="enat", bufs=3))
        stage_pool = ctx.enter_context(tc.tile_pool(name="stage32", bufs=3))
        etr_pool = ctx.enter_context(tc.tile_pool(name="etr", bufs=3))
        work_pool = ctx.enter_context(tc.tile_pool(name="work", bufs=4))
        out_pool = ctx.enter_context(tc.tile_pool(name="outp", bufs=2))
        pa_pool = ctx.enter_context(tc.tile_pool(name="pa", bufs=2))
        ps_t = ctx.enter_context(tc.tile_pool(name="ps_t", bufs=3, space="PSUM"))
        ps_x = ctx.enter_context(tc.tile_pool(name="ps_x", bufs=2, space="PSUM"))
        ps_c = ctx.enter_context(tc.tile_pool(name="ps_c", bufs=1, space="PSUM"))
        ps_s = ctx.enter_context(tc.tile_pool(name="ps_s", bufs=1, space="PSUM"))

        import contextlib
        loop_cm = tc.For_i(0, loop_reps, 1) if loop_reps > 1 else contextlib.nullcontext()
        with loop_cm:
            _batch_body(nc, tc, stage, locals())

    nc.compile()
    return nc


def _batch_body(nc, tc, stage, env):
    enhy_d = env["enhy_d"]; pa_d = env["pa_d"]; cenc_d = env["cenc_d"]
    attn_d = env["attn_d"]; aee_d = env["aee_d"]
    enat_pool = env["enat_pool"]; etr_pool = env["etr_pool"]; work_pool = env["work_pool"]
    stage_pool = env["stage_pool"]
    out_pool = env["out_pool"]; pa_pool = env["pa_pool"]
    ps_t = env["ps_t"]; ps_x = env["ps_x"]; ps_c = env["ps_c"]; ps_s = env["ps_s"]
    wenT = env["wenT"]; bias2 = env["bias2"]; ww_bc = env["ww_bc"]
    maskf = env["maskf"]; negoff = env["negoff"]; scores = env["scores"]
    ones_row = env["ones_row"]; ones_r = env["ones_r"]; ones_col2_r = env["ones_col2_r"]
    id_f = env["id_f"]; id_r = env["id_r"]; id_h = env["id_h"]

    def emit_transposes(ehalves, j):
        eh, jj = ehalves[j // 8], j % 8
        etr = etr_pool.tile([128, DB, 128], F16, tag="etr")
        for hd in range(2):
            pt = ps_t.tile([128, 4, 128], F16, tag="pt")
            for k in range(4):
                db = hd * 4 + k
                nc.tensor.transpose(pt[:, k], eh[:, jj, db * 128:(db + 1) * 128], id_h[:])
            nc.scalar.copy(etr[:, hd * 4:(hd + 1) * 4, :], pt[:])
        return etr

    pending = [None]

    def flush_pending():
        if pending[0] is not None:
            pending[0]()
            pending[0] = None

    for b in range(BPC):
        src = enhy_d[b].rearrange("(j p) d -> p j d", p=128)
        ehalves = []
        for h in range(2):
            eh = enat_pool.tile([128, SB // 2, SRC2], F16, tag="enat")
            for q in range(2):
                lo = h * 8 + q * 4
                st32 = stage_pool.tile([128, 4, SRC2], F32R, tag="st32")
                nc.gpsimd.dma_start(out=st32[:], in_=src[:, lo:lo + 4, :])
                nc.scalar.copy(eh[:, q * 4:(q + 1) * 4, :], st32[:])
            ehalves.append(eh)

        pa2 = pa_pool.tile([2, S], F16, tag="pa2")
        nc.gpsimd.dma_start(out=pa2[0:1, :], in_=pa_d[b:b + 1, :])
        nc.gpsimd.dma_start(out=pa2[1:2, :], in_=ones_row[:])

        if stage == "load":
            continue

        # software pipeline: transposes for j+1 are emitted before matmuls of j;
        # pass 2 + output tail of batch b-1 lands right after this batch's first
        # transpose group so the PE stays busy during b-1's softmax latency.
        etr_cur = emit_transposes(ehalves, 0)
        flush_pending()
        for j in range(SB):
            etr_next = emit_transposes(ehalves, j + 1) if j < SB - 1 else None
            if stage != "transpose":
                px = ps_x.tile([128, TRG], F32, tag="px")
                for db in range(DB):
                    nc.tensor.matmul(px[:], etr_cur[:, db], wenT[:, db],
                                     start=(db == 0), stop=False)
                if stage != "mm":
                    nc.tensor.matmul(px[:], pa2[:, j * 128:(j + 1) * 128], bias2[:, b],
                                     start=False, stop=True)
                else:
                    nc.tensor.matmul(px[:], etr_cur[:, 0], wenT[:, 0], start=False, stop=True)
                tj = work_pool.tile([128, TRG], F32, tag="tanh")
                nc.scalar.activation(tj[:], px[:], AF.Tanh)
                if stage not in ("mm", "mm_rank2"):
                    ttro = work_pool.tile([128, TRG], F32, tag="ttro")
                    nc.vector.tensor_mul(ttro[:], tj[:], ww_bc[:])
                    nc.vector.reduce_sum(scores[:, b * SB + j: b * SB + j + 1], ttro[:],
                                         axis=mybir.AxisListType.X)
            etr_cur = etr_next
        if stage in ("transpose", "scores", "mm", "mm_rank2"):
            continue

        # ---- mask + exp (no max subtraction: |scores| <= sum|w_warp|) ----
        sc_b = scores[:, b * SB:(b + 1) * SB]
        aee = out_pool.tile([128, SB], F32, tag="aee")
        nc.vector.tensor_mul(aee[:], sc_b, maskf[:, :, b])
        nc.vector.tensor_add(aee[:], aee[:], negoff[:, :, b])
        expv = out_pool.tile([128, SB], F32, tag="expv")
        zpart = out_pool.tile([128, 1], F32, tag="zpart")
        nc.scalar.activation(expv[:], aee[:], AF.Exp, accum_out=zpart[:])
        expr = out_pool.tile([128, SB, 2], F16, tag="expr")
        nc.vector.tensor_copy(expr[:, :, 0], expv[:])
        nc.vector.tensor_copy(expr[:, :, 1], expv[:])
        zr = out_pool.tile([128, 1], F32R, tag="zr")
        nc.vector.tensor_copy(zr[:], zpart[:])

        def make_pending(b=b, ehalves=ehalves, aee=aee, expv=expv, expr=expr, zr=zr):
            def emit():
                if stage != "softmax":
                    pc = ps_c.tile([2, SRC2], F32, tag="pc")
                    for j in range(SB):
                        eh, jj = ehalves[j // 8], j % 8
                        for dh in range(2):
                            nc.tensor.matmul(pc[:, dh * 512:(dh + 1) * 512],
                                             expr[:, j],
                                             eh[:, jj, dh * 512:(dh + 1) * 512],
                                             start=(j == 0), stop=(j == SB - 1))
                # normalization chain
                pz = ps_s.tile([128, 128], F32, tag="small")
                nc.tensor.matmul(pz[:1, :2], zr[:], ones_col2_r[:], start=True, stop=True)
                zsb = out_pool.tile([1, 1], F32, tag="zsb")
                nc.vector.tensor_copy(zsb[:], pz[:1, :1])
                rz = out_pool.tile([1, 1], F32, tag="rz")
                nc.vector.reciprocal(rz[:], zsb[:])
                rz2_r = out_pool.tile([1, 2], F32R, tag="rz2_r")
                nc.vector.tensor_copy(rz2_r[:, 0:1], rz[:])
                nc.vector.tensor_copy(rz2_r[:, 1:2], rz[:])
                prz = ps_s.tile([128, 128], F32, tag="small")
                nc.tensor.matmul(prz[:, :2], ones_r[:], rz2_r[:], start=True, stop=True)
                rzcol = out_pool.tile([128, 1], F32, tag="rzcol")
                nc.vector.tensor_copy(rzcol[:], prz[:, :1])
                attn_col = out_pool.tile([128, SB], F32, tag="attn_col")
                nc.vector.tensor_scalar_mul(attn_col[:], expv[:], rzcol[:])

                # outputs: attn_ee / attn (transpose to row layout), c
                pee = ps_s.tile([128, 128], F32, tag="small")
                nc.tensor.transpose(pee[:SB, :], aee[:], id_f[:])
                eeT = out_pool.tile([SB, 128], F32, tag="eeT")
                nc.vector.tensor_copy(eeT[:], pee[:SB, :])
                nc.gpsimd.dma_start(out=aee_d[b].rearrange("(j c) -> j c", j=SB), in_=eeT[:])
                patn = ps_s.tile([128, 128], F32, tag="small")
                nc.tensor.transpose(patn[:SB, :], attn_col[:], id_f[:])
                attnT = out_pool.tile([SB, 128], F32, tag="attnT")
                nc.vector.tensor_copy(attnT[:], patn[:SB, :])
                nc.gpsimd.dma_start(out=attn_d[b].rearrange("(j c) -> j c", j=SB), in_=attnT[:])
                if stage != "softmax":
                    csb = out_pool.tile([1, SRC2], F32, tag="csb")
                    nc.scalar.activation(csb[:], pc[:1, :], AF.Copy, scale=rz[:])
                    nc.gpsimd.dma_start(out=cenc_d[b:b + 1, :], in_=csb[:])
            return emit

        pending[0] = make_pending()
    flush_pending()


_NC_CACHE = None


def _get_nc():
    global _NC_CACHE
    if _NC_CACHE is None:
        _NC_CACHE = build_nc()
    return _NC_CACHE


def kernel(dehy, enhy, past_attn, src_mask, W_en, b_en, W_de, w_cv, w_warp):
    dehy = np.ascontiguousarray(np.asarray(dehy, dtype=np.float32))
    enhy = np.ascontiguousarray(np.asarray(enhy, dtype=np.float32))
    past_attn = np.ascontiguousarray(np.asarray(past_attn, dtype=np.float32))
    src_mask = np.ascontiguousarray(np.asarray(src_mask, dtype=np.int32))
    W_en = np.ascontiguousarray(np.asarray(W_en, dtype=np.float32))
    b_en = np.ascontiguousarray(np.asarray(b_en, dtype=np.float32))
    W_de = np.ascontiguousarray(np.asarray(W_de, dtype=np.float32))
    w_cv = np.ascontiguousarray(np.asarray(w_cv, dtype=np.float32))
    w_warp = np.ascontiguousarray(np.asarray(w_warp, dtype=np.float32))

    nc = _get_nc()
    in_maps = []
    for c in range(N_CORES):
        sl = slice(c * BPC, (c + 1) * BPC)
        in_maps.append({
            "dehy": dehy[sl], "enhy": enhy[sl], "past_attn": past_attn[sl],
            "src_mask": src_mask[sl], "W_en": W_en, "b_en": b_en,
            "W_de": W_de, "w_cv": w_cv, "w_warp": w_warp,
        })
    res = run_bass_kernel_spmd(nc, in_maps, core_ids=list(range(N_CORES)))
    c_encoder = np.concatenate([res.results[c]["c_encoder"] for c in range(N_CORES)], axis=0)
    attn = np.concatenate([res.results[c]["attn"] for c in range(N_CORES)], axis=0)
    attn_ee = np.concatenate([res.results[c]["attn_ee"] for c in range(N_CORES)], axis=0)
    return (c_encoder, attn, attn_ee)


if __name__ == "__main__":
    rng = np.random.default_rng(0)
    outs = kernel(
        dehy=rng.standard_normal((B, TRG)).astype(np.float32),
        enhy=rng.standard_normal((B, S, SRC2)).astype(np.float32),
        past_attn=rng.random((B, S)).astype(np.float32),
        src_mask=rng.integers(0, 2, (B, S)).astype(np.int32),
        W_en=(rng.standard_normal((TRG, SRC2)) * 0.02).astype(np.float32),
        b_en=np.zeros((TRG,), np.float32),
        W_de=(rng.standard_normal((TRG, TRG)) * 0.02).astype(np.float32),
        w_cv=(rng.standard_normal((TRG,)) * 0.02).astype(np.float32),
        w_warp=(rng.standard_normal((TRG,)) * 0.02).astype(np.float32),
    )
    for name, o in zip(["c_encoder", "attn", "attn_ee"], outs):
        print(name, o.shape, o.dtype, float(np.abs(o).mean()))
